# revision 35
# baseline (speedup 1.0000x reference)
"""Q8 linear layer (dequant matmul) on 8 Trainium2 NeuronCores.

out[t, o] = sum_i (x[t, i] * scales[i]) * weight[o, i]

Sharding: tensor-parallel over out_features (14336 = 8 * 1792). Each core
gets the full pre-scaled activations and a 1792-column slice of weight^T.

Strategy (memory-bound; weight HBM traffic is the floor — ~358 GB/s/core
with 8 cores sharing HBM, so the 7.34 MB int8 stream is ~20 us):
  - Ship weights as 1 B/elem in a host-prearranged layout that is
    contiguous per SBUF partition (7168 B descriptors, HWDGE SP ring).
    Columns [0, 1344) are packed two-per-uint16 (biased to unsigned, pair
    (m, 672+m)); columns [1344, 1792) are plain int8. The last two DMA
    groups are fully packed so the post-stream tail is DVE-only.
  - DVE unpacks the packed region with two tensor_scalar ops per chunk
    ((w & 0xFF) | 0x6400 and (w >> 8) | 0x6400), which run in the 4x
    perf mode (16-bit dtype) and materialize fp16 "magic" values
    1024 + (w + 128) = w + 1152 directly as bit patterns. ACT converts the
    plain int8 region with activation-Copy. Combined conversion rate
    exceeds the DMA rate, so the kernel stays DMA-bound.
  - fp16 matmul (x pre-scaled to fp16 on host), packed4 col-groups
    (tile_position), accumulating into 4 PSUM banks [128, 448] over all
    32 k-tiles.
  - PSUM is evacuated as fp16 partials with the +1152 magic bias removed
    on-chip (per-partition bias vector, DVE tensor_scalar / ACT
    activation-Identity), then leaves in ONE partition-major DMA; the
    host folds the 4 col-group strips.
"""

import os
import sys

for _p in ("/opt/trn_rl_repo", "/root/.axon_site/_ro/trn_rl_repo"):
    if os.path.isdir(_p) and _p not in sys.path:
        sys.path.insert(0, _p)

import numpy as np

import concourse.bass as bass
import concourse.mybir as mybir
import concourse.tile as tile
from concourse import bacc
from concourse.bass_utils import run_bass_kernel_spmd

TOKENS = 32
IN_F = 4096
OUT_F = 14336
NCORES = 8
OPC = OUT_F // NCORES  # 1792 out features per core
P = 128
KT = IN_F // P  # 32 k-tiles
OB = 4
OBS = OPC // OB  # 448 (one PSUM bank)

PACK_COLS = 1344  # columns converted via DVE uint16 bit-trick (w + 1152)
PACK_W = PACK_COLS // 2  # 672 uint16 words per k-tile
PLAIN_COLS = OPC - PACK_COLS  # 448 columns converted via ACT int8 copy
KB = OPC  # bytes per k-tile per partition (1344 packed + 448 plain)

GROUPS = [4, 4, 4, 4, 4, 4, 4, 2, 2]  # k-tiles per DMA group, sum = 32
assert sum(GROUPS) == KT
# groups whose k-tiles are FULLY packed as uint16 pairs (DVE-only conversion,
# no ACT work) so the kernel tail after the last DMA is as short as possible
FULL_PACK_GROUPS = {7, 8}
FP_KSTART = 28  # first k-tile index of the fully packed groups
PACK_W1 = PLAIN_COLS // 2  # 224 words in the second pack unit

_cached_nc = {}


def _build():
    key = 0
    if key in _cached_nc:
        return _cached_nc[key]

    nc = bacc.Bacc(
        "TRN2", target_bir_lowering=False, debug=False, num_devices=NCORES
    )
    xsT = nc.dram_tensor(
        "xsT", [P, KT, TOKENS], mybir.dt.float16, kind="ExternalInput"
    )
    w8 = nc.dram_tensor("w8", [P, KT * KB], mybir.dt.uint8, kind="ExternalInput")
    # negated per-partition bias (1152 * partial rowsums of xs): col 0 for the
    # always-packed cols 0:1344 (ob 0-2), col 1 for cols 1344:1792 (ob 3)
    bias = nc.dram_tensor("bias", [P, 2], mybir.dt.float32, kind="ExternalInput")
    # partition-major output: one descriptor per partition, one DMA total
    outp = nc.dram_tensor(
        "outp", [P, OB * OBS], mybir.dt.float16, kind="ExternalOutput"
    )

    gstart = [sum(GROUPS[:i]) for i in range(len(GROUPS))]  # first k-tile of group
    w8_flat = w8.ap()

    with tile.TileContext(nc) as tc:
        with (
            tc.tile_pool(name="xpool", bufs=1) as xpool,
            tc.tile_pool(name="w8pool", bufs=len(GROUPS)) as w8pool,
            tc.tile_pool(name="w16pool", bufs=8) as w16pool,
            tc.tile_pool(name="opool", bufs=1) as opool,
            tc.tile_pool(name="pspool", bufs=1, space=bass.MemorySpace.PSUM) as pspool,
        ):
            xs_sb = xpool.tile([P, KT, TOKENS], mybir.dt.float16, name="xs_sb")
            w8_tiles = []
            for g, kg in enumerate(GROUPS):
                t8 = w8pool.tile([P, kg, KB], mybir.dt.uint8, name=f"w8_{g}", tag="w8")
                src = w8_flat[:, gstart[g] * KB : (gstart[g] + kg) * KB]
                nc.sync.dma_start(out=t8[:], in_=src)
                w8_tiles.append(t8)
                if g == 0:
                    # xs queued right behind the first weight group: the weight
                    # stream starts immediately, xs lands well before round 0
                    nc.sync.dma_start(out=xs_sb[:], in_=xsT.ap())
                    bias_sb = xpool.tile([P, 2], mybir.dt.float32, name="bias_sb")
                    nc.sync.dma_start(out=bias_sb[:], in_=bias.ap())

            psums = [
                pspool.tile([P, OBS], mybir.dt.float32, name=f"ps_{ob}", tag=f"ps{ob}")
                for ob in range(OB)
            ]

            w16_tiles = []
            for g, kg in enumerate(GROUPS):
                t8 = w8_tiles[g]
                t16 = w16pool.tile(
                    [P, kg, OPC], mybir.dt.uint16, name=f"w16_{g}", tag="w16"
                )
                def unpack(out_ap, in_ap, hi):
                    nc.vector.tensor_scalar(
                        out_ap,
                        in_ap,
                        8 if hi else 0x00FF,
                        0x6400,
                        mybir.AluOpType.logical_shift_right
                        if hi
                        else mybir.AluOpType.bitwise_and,
                        mybir.AluOpType.bitwise_or,
                    )

                if g in FULL_PACK_GROUPS:
                    # all 1792 cols packed: unit0 = 672 words -> cols 0:1344,
                    # unit1 = 224 words -> cols 1344:1792; per k-tile chunks
                    for k0 in range(kg):
                        k1 = k0 + 1
                        pk0 = t8[:, k0:k1, 0:PACK_COLS].bitcast(mybir.dt.uint16)
                        pk1 = t8[:, k0:k1, PACK_COLS:OPC].bitcast(mybir.dt.uint16)
                        unpack(t16[:, k0:k1, 0:PACK_W], pk0, False)
                        unpack(t16[:, k0:k1, PACK_W:PACK_COLS], pk0, True)
                        unpack(t16[:, k0:k1, PACK_COLS : PACK_COLS + PACK_W1], pk1, False)
                        unpack(t16[:, k0:k1, PACK_COLS + PACK_W1 : OPC], pk1, True)
                else:
                    # cols 0:1344 packed on DVE (k-pair chunks), cols 1344:1792
                    # plain int8 on ACT (whole group)
                    for k0 in range(0, kg, 2):
                        k1 = k0 + 2
                        pk = t8[:, k0:k1, 0:PACK_COLS].bitcast(mybir.dt.uint16)
                        unpack(t16[:, k0:k1, 0:PACK_W], pk, False)
                        unpack(t16[:, k0:k1, PACK_W:PACK_COLS], pk, True)
                    nc.scalar.copy(
                        t16[:, :, PACK_COLS:OPC].bitcast(mybir.dt.float16),
                        t8[:, :, PACK_COLS:OPC].bitcast(mybir.dt.int8),
                    )
                w16_tiles.append(t16)

            # matmul rounds: round r consumes k-tiles 4r+j in col-group j
            def w16_ap(ki, ob):
                # locate group containing k-tile ki
                for g, kg in enumerate(GROUPS):
                    if gstart[g] <= ki < gstart[g] + kg:
                        return w16_tiles[g][
                            :, ki - gstart[g], ob * OBS : (ob + 1) * OBS
                        ].bitcast(mybir.dt.float16)
                raise AssertionError(ki)

            out_sb = opool.tile([P, OB, OBS], mybir.dt.float16, name="out_sb")

            def evac(ob):
                # fp16 partials with the magic bias subtracted on-chip
                # (bias col 0 for ob 0-2, col 1 for ob 3)
                b_ap = bias_sb[:, (1 if ob == 3 else 0) : (2 if ob == 3 else 1)]
                if ob in (1, 2):
                    nc.scalar.activation(
                        out_sb[:, ob, :],
                        psums[ob][:, :],
                        mybir.ActivationFunctionType.Identity,
                        bias=b_ap,
                    )
                else:
                    nc.vector.tensor_scalar(
                        out_sb[:, ob, :],
                        psums[ob][:, :],
                        b_ap,
                        None,
                        mybir.AluOpType.add,
                    )

            def mm(r, j, ob):
                nc.tensor.matmul(
                    psums[ob][32 * j : 32 * (j + 1), :],
                    xs_sb[:, 4 * r + j, :],
                    w16_ap(4 * r + j, ob),
                    start=(r == 0),
                    stop=(r == nrounds - 1),
                    tile_position=(0, 32 * j),
                    # sim's zero-region group check drops the partition
                    # base of col-group strips; disjoint strips are safe
                    skip_group_check=True,
                )

            nrounds = KT // 4
            for r in range(nrounds - 1):
                for j in range(4):
                    for ob in range(OB):
                        mm(r, j, ob)
            # final round ob-major: each PSUM bank closes as early as possible
            # and is evacuated immediately; output leaves in two
            # partition-major DMAs so the first transfer overlaps the
            # remaining evacuations
            for ob in range(OB):
                for j in range(4):
                    mm(nrounds - 1, j, ob)
                evac(ob)
                if ob == 1:
                    nc.sync.dma_start(
                        out=outp.ap()[:, 0 : 2 * OBS], in_=out_sb[:, 0:2, :]
                    )
            nc.sync.dma_start(
                out=outp.ap()[:, 2 * OBS : 4 * OBS], in_=out_sb[:, 2:4, :]
            )

    nc.compile()
    _cached_nc[key] = nc
    return nc


def make_in_maps(x, weight, scales):
    x = np.asarray(x, dtype=np.float32)
    weight = np.asarray(weight)
    scales = np.asarray(scales, dtype=np.float32)
    assert x.shape == (TOKENS, IN_F) and weight.shape == (OUT_F, IN_F)

    xs = x * scales[None, :]
    # [P, KT, TOKENS]: xsT[p, nk, t] = xs[t, nk*128 + p]
    xsT = np.ascontiguousarray(
        xs.T.reshape(KT, P, TOKENS).transpose(1, 0, 2)
    ).astype(np.float16)

    # negated magic-bias per psum partition 32j+t: col-group j accumulates
    # k-tiles {4r+j}; packed cells contribute 1152 * xs per element
    xs16 = xsT.astype(np.float32)  # [P, KT, T]
    ksum = xs16.sum(axis=0).T  # [T, KT] per-k-tile rowsums
    bA = np.zeros((4, TOKENS), dtype=np.float32)
    bB = np.zeros((4, TOKENS), dtype=np.float32)
    for j in range(4):
        bA[j] = 1152.0 * ksum[:, [4 * r + j for r in range(KT // 4)]].sum(axis=1)
        bB[j] = 1152.0 * ksum[:, FP_KSTART + j]
    bias = np.stack([-bA.reshape(P), -bB.reshape(P)], axis=1).astype(
        np.float32
    )  # [128, 2]

    u8_full = (weight.astype(np.int16) + 128).astype(np.uint8)  # biased weights
    i8_full = weight.astype(np.int8)
    in_maps = []
    for c in range(NCORES):
        su = u8_full[c * OPC : (c + 1) * OPC, :]  # [OPC, IN_F] biased
        si = i8_full[c * OPC : (c + 1) * OPC, :]
        sut = su.T.reshape(KT, P, OPC)  # [KT, P, n]
        sit = si.T.reshape(KT, P, OPC)

        def pack_pairs(a):  # a: [..., 2*W] biased -> packed bytes [..., 2*W]
            w = a.shape[-1] // 2
            lo = a[..., 0:w].astype(np.uint16)
            hi = a[..., w : 2 * w].astype(np.uint16)
            return np.ascontiguousarray(lo | (hi << 8)).view(np.uint8)

        # k-tiles < FP_KSTART: 1344 packed bytes + 448 plain int8
        # k-tiles >= FP_KSTART: fully packed (unit0 1344 B + unit1 448 B)
        head = np.concatenate(
            [
                pack_pairs(sut[:FP_KSTART, :, 0:PACK_COLS]),
                sit[:FP_KSTART, :, PACK_COLS:OPC].view(np.uint8),
            ],
            axis=2,
        )
        tail = np.concatenate(
            [
                pack_pairs(sut[FP_KSTART:, :, 0:PACK_COLS]),
                pack_pairs(sut[FP_KSTART:, :, PACK_COLS:OPC]),
            ],
            axis=2,
        )
        blob = np.concatenate([head, tail], axis=0)  # [KT, P, KB]
        w8c = np.ascontiguousarray(blob.transpose(1, 0, 2)).reshape(P, KT * KB)
        in_maps.append({"xsT": xsT, "w8": w8c, "bias": bias})
    return in_maps


def run(x, weight, scales, trace=False, trace_cores=None):
    nc = _build()
    in_maps = make_in_maps(x, weight, scales)
    res = run_bass_kernel_spmd(
        nc,
        in_maps,
        core_ids=list(range(NCORES)),
        trace=trace,
        trace_cores=trace_cores,
    )
    cols = []
    for c in range(NCORES):
        part = (
            res.results[c]["outp"]
            .astype(np.float32)
            .reshape(4, TOKENS, OB, OBS)  # partition 32j+t -> (j, t)
        )
        folded = part.sum(axis=0)  # [TOKENS, OB, OBS]
        cols.append(folded.reshape(TOKENS, OPC))
    out = np.concatenate(cols, axis=1).astype(np.float32, copy=False)
    return out, res


def kernel(x, weight, scales):
    out, _ = run(x, weight, scales)
    return out


# revision 36
# speedup vs baseline: 1.0441x; 1.0441x over previous
"""Q8 linear layer (dequant matmul) on 8 Trainium2 NeuronCores.

out[t, o] = sum_i (x[t, i] * scales[i]) * weight[o, i]

Sharding: tensor-parallel over out_features (14336 = 8 * 1792). Each core
gets the full pre-scaled activations and a 1792-column slice of weight^T.

Strategy (memory-bound; weight HBM traffic is the floor — ~358 GB/s/core
with 8 cores sharing HBM, so the 7.34 MB int8 stream is ~20 us):
  - Ship weights as 1 B/elem in a host-prearranged layout that is
    contiguous per SBUF partition (7168 B descriptors, HWDGE SP ring).
    Columns [0, 1344) are packed two-per-uint16 (biased to unsigned, pair
    (m, 672+m)); columns [1344, 1792) are plain int8. The last two DMA
    groups are fully packed so the post-stream tail is DVE-only.
  - DVE unpacks the packed region with two tensor_scalar ops per chunk
    ((w & 0xFF) | 0x6400 and (w >> 8) | 0x6400), which run in the 4x
    perf mode (16-bit dtype) and materialize fp16 "magic" values
    1024 + (w + 128) = w + 1152 directly as bit patterns. ACT converts the
    plain int8 region with activation-Copy. Combined conversion rate
    exceeds the DMA rate, so the kernel stays DMA-bound.
  - fp16 matmul (x pre-scaled to fp16 on host), packed4 col-groups
    (tile_position), accumulating into 4 PSUM banks [128, 448] over all
    32 k-tiles.
  - The final round runs ob-major so each PSUM bank closes early and is
    evacuated immediately as fp16 partials with the +1152 magic bias
    removed on-chip (per-partition bias vector, DVE tensor_scalar / ACT
    activation-Identity). Output leaves in two partition-major DMAs so
    the first transfer overlaps the remaining evacuations; the host folds
    the 4 col-group strips.
"""

import os
import sys

for _p in ("/opt/trn_rl_repo", "/root/.axon_site/_ro/trn_rl_repo"):
    if os.path.isdir(_p) and _p not in sys.path:
        sys.path.insert(0, _p)

import numpy as np

import concourse.bass as bass
import concourse.mybir as mybir
import concourse.tile as tile
from concourse import bacc
from concourse.bass_utils import run_bass_kernel_spmd

TOKENS = 32
IN_F = 4096
OUT_F = 14336
NCORES = 8
OPC = OUT_F // NCORES  # 1792 out features per core
P = 128
KT = IN_F // P  # 32 k-tiles
OB = 4
OBS = OPC // OB  # 448 (one PSUM bank)

PACK_COLS = 1344  # columns converted via DVE uint16 bit-trick (w + 1152)
PACK_W = PACK_COLS // 2  # 672 uint16 words per k-tile
PLAIN_COLS = OPC - PACK_COLS  # 448 columns converted via ACT int8 copy
KB = OPC  # bytes per k-tile per partition (1344 packed + 448 plain)

GROUPS = [4, 4, 4, 4, 4, 4, 4, 2, 2]  # k-tiles per DMA group, sum = 32
assert sum(GROUPS) == KT
# groups whose k-tiles are FULLY packed as uint16 pairs (DVE-only conversion,
# no ACT work) so the kernel tail after the last DMA is as short as possible
FULL_PACK_GROUPS = {7, 8}
FP_KSTART = 28  # first k-tile index of the fully packed groups
PACK_W1 = PLAIN_COLS // 2  # 224 words in the second pack unit

_cached_nc = {}


def _build():
    key = 0
    if key in _cached_nc:
        return _cached_nc[key]

    nc = bacc.Bacc(
        "TRN2", target_bir_lowering=False, debug=False, num_devices=NCORES
    )
    xsT = nc.dram_tensor(
        "xsT", [P, KT, TOKENS], mybir.dt.float16, kind="ExternalInput"
    )
    w8 = nc.dram_tensor("w8", [P, KT * KB], mybir.dt.uint8, kind="ExternalInput")
    # negated per-partition bias (1152 * partial rowsums of xs): col 0 for the
    # always-packed cols 0:1344 (ob 0-2), col 1 for cols 1344:1792 (ob 3)
    bias = nc.dram_tensor("bias", [P, 2], mybir.dt.float32, kind="ExternalInput")
    # partition-major output: one descriptor per partition, one DMA total
    outp = nc.dram_tensor(
        "outp", [P, OB * OBS], mybir.dt.float16, kind="ExternalOutput"
    )

    gstart = [sum(GROUPS[:i]) for i in range(len(GROUPS))]  # first k-tile of group
    w8_flat = w8.ap()

    with tile.TileContext(nc) as tc:
        with (
            tc.tile_pool(name="xpool", bufs=1) as xpool,
            tc.tile_pool(name="w8pool", bufs=len(GROUPS)) as w8pool,
            tc.tile_pool(name="w16pool", bufs=8) as w16pool,
            tc.tile_pool(name="opool", bufs=1) as opool,
            tc.tile_pool(name="pspool", bufs=1, space=bass.MemorySpace.PSUM) as pspool,
        ):
            xs_sb = xpool.tile([P, KT, TOKENS], mybir.dt.float16, name="xs_sb")
            w8_tiles = []
            for g, kg in enumerate(GROUPS):
                t8 = w8pool.tile([P, kg, KB], mybir.dt.uint8, name=f"w8_{g}", tag="w8")
                src = w8_flat[:, gstart[g] * KB : (gstart[g] + kg) * KB]
                nc.sync.dma_start(out=t8[:], in_=src)
                w8_tiles.append(t8)
                if g == 0:
                    # xs queued right behind the first weight group: the weight
                    # stream starts immediately, xs lands well before round 0
                    nc.sync.dma_start(out=xs_sb[:], in_=xsT.ap())
                    bias_sb = xpool.tile([P, 2], mybir.dt.float32, name="bias_sb")
                    nc.sync.dma_start(out=bias_sb[:], in_=bias.ap())

            psums = [
                pspool.tile([P, OBS], mybir.dt.float32, name=f"ps_{ob}", tag=f"ps{ob}")
                for ob in range(OB)
            ]

            w16_tiles = []
            for g, kg in enumerate(GROUPS):
                t8 = w8_tiles[g]
                t16 = w16pool.tile(
                    [P, kg, OPC], mybir.dt.uint16, name=f"w16_{g}", tag="w16"
                )
                def unpack(out_ap, in_ap, hi):
                    nc.vector.tensor_scalar(
                        out_ap,
                        in_ap,
                        8 if hi else 0x00FF,
                        0x6400,
                        mybir.AluOpType.logical_shift_right
                        if hi
                        else mybir.AluOpType.bitwise_and,
                        mybir.AluOpType.bitwise_or,
                    )

                if g in FULL_PACK_GROUPS:
                    # all 1792 cols packed: unit0 = 672 words -> cols 0:1344,
                    # unit1 = 224 words -> cols 1344:1792; per k-tile chunks
                    for k0 in range(kg):
                        k1 = k0 + 1
                        pk0 = t8[:, k0:k1, 0:PACK_COLS].bitcast(mybir.dt.uint16)
                        pk1 = t8[:, k0:k1, PACK_COLS:OPC].bitcast(mybir.dt.uint16)
                        unpack(t16[:, k0:k1, 0:PACK_W], pk0, False)
                        unpack(t16[:, k0:k1, PACK_W:PACK_COLS], pk0, True)
                        unpack(t16[:, k0:k1, PACK_COLS : PACK_COLS + PACK_W1], pk1, False)
                        unpack(t16[:, k0:k1, PACK_COLS + PACK_W1 : OPC], pk1, True)
                else:
                    # cols 0:1344 packed on DVE (k-pair chunks), cols 1344:1792
                    # plain int8 on ACT (whole group)
                    for k0 in range(0, kg, 2):
                        k1 = k0 + 2
                        pk = t8[:, k0:k1, 0:PACK_COLS].bitcast(mybir.dt.uint16)
                        unpack(t16[:, k0:k1, 0:PACK_W], pk, False)
                        unpack(t16[:, k0:k1, PACK_W:PACK_COLS], pk, True)
                    nc.scalar.copy(
                        t16[:, :, PACK_COLS:OPC].bitcast(mybir.dt.float16),
                        t8[:, :, PACK_COLS:OPC].bitcast(mybir.dt.int8),
                    )
                w16_tiles.append(t16)

            # matmul rounds: round r consumes k-tiles 4r+j in col-group j
            def w16_ap(ki, ob):
                # locate group containing k-tile ki
                for g, kg in enumerate(GROUPS):
                    if gstart[g] <= ki < gstart[g] + kg:
                        return w16_tiles[g][
                            :, ki - gstart[g], ob * OBS : (ob + 1) * OBS
                        ].bitcast(mybir.dt.float16)
                raise AssertionError(ki)

            out_sb = opool.tile([P, OB, OBS], mybir.dt.float16, name="out_sb")

            def evac(ob):
                # fp16 partials with the magic bias subtracted on-chip
                # (bias col 0 for ob 0-2, col 1 for ob 3)
                b_ap = bias_sb[:, (1 if ob == 3 else 0) : (2 if ob == 3 else 1)]
                if ob in (1, 2):
                    nc.scalar.activation(
                        out_sb[:, ob, :],
                        psums[ob][:, :],
                        mybir.ActivationFunctionType.Identity,
                        bias=b_ap,
                    )
                else:
                    nc.vector.tensor_scalar(
                        out_sb[:, ob, :],
                        psums[ob][:, :],
                        b_ap,
                        None,
                        mybir.AluOpType.add,
                    )

            def mm(r, j, ob):
                nc.tensor.matmul(
                    psums[ob][32 * j : 32 * (j + 1), :],
                    xs_sb[:, 4 * r + j, :],
                    w16_ap(4 * r + j, ob),
                    start=(r == 0),
                    stop=(r == nrounds - 1),
                    tile_position=(0, 32 * j),
                    # sim's zero-region group check drops the partition
                    # base of col-group strips; disjoint strips are safe
                    skip_group_check=True,
                )

            nrounds = KT // 4
            for r in range(nrounds - 1):
                for j in range(4):
                    for ob in range(OB):
                        mm(r, j, ob)
            # final round ob-major: each PSUM bank closes as early as possible
            # and is evacuated immediately; output leaves in two
            # partition-major DMAs so the first transfer overlaps the
            # remaining evacuations
            for ob in range(OB):
                for j in range(4):
                    mm(nrounds - 1, j, ob)
                evac(ob)
                if ob == 1:
                    nc.sync.dma_start(
                        out=outp.ap()[:, 0 : 2 * OBS], in_=out_sb[:, 0:2, :]
                    )
            nc.sync.dma_start(
                out=outp.ap()[:, 2 * OBS : 4 * OBS], in_=out_sb[:, 2:4, :]
            )

    nc.compile()
    _cached_nc[key] = nc
    return nc


def make_in_maps(x, weight, scales):
    x = np.asarray(x, dtype=np.float32)
    weight = np.asarray(weight)
    scales = np.asarray(scales, dtype=np.float32)
    assert x.shape == (TOKENS, IN_F) and weight.shape == (OUT_F, IN_F)

    xs = x * scales[None, :]
    # [P, KT, TOKENS]: xsT[p, nk, t] = xs[t, nk*128 + p]
    xsT = np.ascontiguousarray(
        xs.T.reshape(KT, P, TOKENS).transpose(1, 0, 2)
    ).astype(np.float16)

    # negated magic-bias per psum partition 32j+t: col-group j accumulates
    # k-tiles {4r+j}; packed cells contribute 1152 * xs per element
    xs16 = xsT.astype(np.float32)  # [P, KT, T]
    ksum = xs16.sum(axis=0).T  # [T, KT] per-k-tile rowsums
    bA = np.zeros((4, TOKENS), dtype=np.float32)
    bB = np.zeros((4, TOKENS), dtype=np.float32)
    for j in range(4):
        bA[j] = 1152.0 * ksum[:, [4 * r + j for r in range(KT // 4)]].sum(axis=1)
        bB[j] = 1152.0 * ksum[:, FP_KSTART + j]
    bias = np.stack([-bA.reshape(P), -bB.reshape(P)], axis=1).astype(
        np.float32
    )  # [128, 2]

    u8_full = (weight.astype(np.int16) + 128).astype(np.uint8)  # biased weights
    i8_full = weight.astype(np.int8)
    in_maps = []
    for c in range(NCORES):
        su = u8_full[c * OPC : (c + 1) * OPC, :]  # [OPC, IN_F] biased
        si = i8_full[c * OPC : (c + 1) * OPC, :]
        sut = su.T.reshape(KT, P, OPC)  # [KT, P, n]
        sit = si.T.reshape(KT, P, OPC)

        def pack_pairs(a):  # a: [..., 2*W] biased -> packed bytes [..., 2*W]
            w = a.shape[-1] // 2
            lo = a[..., 0:w].astype(np.uint16)
            hi = a[..., w : 2 * w].astype(np.uint16)
            return np.ascontiguousarray(lo | (hi << 8)).view(np.uint8)

        # k-tiles < FP_KSTART: 1344 packed bytes + 448 plain int8
        # k-tiles >= FP_KSTART: fully packed (unit0 1344 B + unit1 448 B)
        head = np.concatenate(
            [
                pack_pairs(sut[:FP_KSTART, :, 0:PACK_COLS]),
                sit[:FP_KSTART, :, PACK_COLS:OPC].view(np.uint8),
            ],
            axis=2,
        )
        tail = np.concatenate(
            [
                pack_pairs(sut[FP_KSTART:, :, 0:PACK_COLS]),
                pack_pairs(sut[FP_KSTART:, :, PACK_COLS:OPC]),
            ],
            axis=2,
        )
        blob = np.concatenate([head, tail], axis=0)  # [KT, P, KB]
        w8c = np.ascontiguousarray(blob.transpose(1, 0, 2)).reshape(P, KT * KB)
        in_maps.append({"xsT": xsT, "w8": w8c, "bias": bias})
    return in_maps


def run(x, weight, scales, trace=False, trace_cores=None):
    nc = _build()
    in_maps = make_in_maps(x, weight, scales)
    res = run_bass_kernel_spmd(
        nc,
        in_maps,
        core_ids=list(range(NCORES)),
        trace=trace,
        trace_cores=trace_cores,
    )
    cols = []
    for c in range(NCORES):
        part = (
            res.results[c]["outp"]
            .astype(np.float32)
            .reshape(4, TOKENS, OB, OBS)  # partition 32j+t -> (j, t)
        )
        folded = part.sum(axis=0)  # [TOKENS, OB, OBS]
        cols.append(folded.reshape(TOKENS, OPC))
    out = np.concatenate(cols, axis=1).astype(np.float32, copy=False)
    return out, res


def kernel(x, weight, scales):
    out, _ = run(x, weight, scales)
    return out
